# revision 13
# baseline (speedup 1.0000x reference)
"""Trainium2 Bass kernel for FeatureTransformerSlice (embedding lookup), v3.

out[b, :] = bias + sum_f mask(idx[b,f]) * val[b,f] * weight[max(idx[b,f],0), :]

Strategy (8 NeuronCores, data-parallel over batch):
  - Per-core work: random gather of 2048*32 = 64Ki table rows + rank-1
    combines.  The binding resource is SWDGE descriptor generation on the
    GpSimd Q7 cores (~2.6 ns/row aggregate over the 4 SWDGE queues; each
    queue's descgen runs on its own Q7 core pair, cpu 2q/2q+1).  dma_gather
    calls are issued round-robin over all 4 queues; js=8 (1024 idxs) is the
    largest call that fits the 64-descriptor/engine single-packet limit.
  - The table is cast host-side to fp8 e3m4 ("float8e3", scaled by 2^11 so
    values sit in the normal range; the 2^-11 rides the bf16 vals exactly),
    halving SDMA drain time vs bf16 so descgen stays the only bottleneck.
    rel-err lands at 1.58e-2 (budget 2e-2), dominated by the 4-bit mantissa.
  - dma_gather indices are int16, so features are split between two
    overlapping table windows: A = w[0:32768] and B = w[8192:40960]
    (local idx - 8192).  Features in the overlap ride either window, so per
    tile J_B = max must-B count and J_A = 32 - J_B pack every row's 32
    features with zero padding.
  - v3 packs the gather calls ACROSS tile boundaries: all tiles' A-slots
    form one slot stream (tile-major), ditto B; calls take uniform js=8
    bites from each stream, so there are ~65 calls instead of 84 and no
    tiny fragment calls.  num_idxs registers are hoisted (no per-call MOVE).
  - Per batch tile: gathered [128, *, 512] fp8 rows are combined on PE as
    diag(val_j) matmuls (bf16 lhsT x fp8 rhs) accumulating in fp32 PSUM;
    Scalar evacuates PSUM as bf16; bias is added host-side.  ~48 warm-up
    matmuls bridge the initial gather fill so the PE reaches 2.4 GHz early.
"""

import numpy as np
import ml_dtypes

P = 128
B = 16384
F = 32
V = 40960
O = 512
NCORES = 8
ABASE = 0           # window A = rows [0, 32768)
BBASE = V - 32768   # window B = rows [8192, 40960)
AEND = 32768

WDT = "float8e3"          # device table dtype (e3m4; rows scaled by 2^WEXP)
WEXP = 11                 # w stored as w*2^WEXP (max |w|*2^11 ~ 10.1 < 15.5)
GBUFS = 32                # gather call ring depth
JSUB = 8                  # slots per dma_gather call (64 descs/engine limit)
OUT_BF16 = True           # device writes bf16 output; host upcasts
NQ = 4                    # SWDGE queues (max 4); round-robin
NWARM = 48                # PE warm-up matmuls
JD = 6                    # per-tile slots combined on DVE (FMA) instead of PE


def _plan_calls(JA, JB):
    """Chunk the tile-major A and B slot streams into uniform js<=JSUB calls.

    Returns (callplan, slot2call, JA) where callplan is a list of
    (stream, stream_slot_off, js, t0) in emission order (grouped by the
    first tile each call serves), slot2call maps (stream, stream_slot) ->
    (call_index, j), and JA may have its last entry padded by one slot to
    keep the total A-slot count even (32B idx alignment for the B stream).
    """
    JA = list(JA)
    if sum(JA) % 2:
        JA[-1] += 1  # pad slot (idx 0, val 0) keeps B stream 32B-aligned
    tile_of = {0: [], 1: []}
    for g in range(len(JA)):
        tile_of[0] += [g] * JA[g]
        tile_of[1] += [g] * JB[g]
    calls = {}
    for s in (0, 1):
        n = len(tile_of[s])
        offs = list(range(0, n, JSUB))
        calls[s] = [(s, o, min(JSUB, n - o), tile_of[s][o]) for o in offs]
    # emission order: by first-served tile, A before B
    plan = sorted(calls[0] + calls[1], key=lambda c: (c[3], c[0]))
    slot2call = {}
    for ci, (s, o, js, _) in enumerate(plan):
        for j in range(js):
            slot2call[(s, o + j)] = (ci, j)
    return plan, slot2call, tuple(JA)


def build_kernel(JA, JB, wdt_name=WDT, v=V, o=O):
    import concourse.bacc as bacc
    import concourse.mybir as mybir
    import concourse.tile as tile

    f32 = mybir.dt.float32
    bf16 = mybir.dt.bfloat16
    fp16 = mybir.dt.float16
    i16 = mybir.dt.int16
    wdt = getattr(mybir.dt, wdt_name)
    tiles = len(JA)
    assert len(JB) == tiles

    plan, slot2call, JA = _plan_calls(JA, JB)
    SA, SB = sum(JA), sum(JB)
    S = SA + SB
    IDXW = S * 8  # int16 columns; A stream then B stream

    nc = bacc.Bacc("TRN2", target_bir_lowering=False, debug=False,
                   num_swdge_queues=NQ)

    idx_d = nc.dram_tensor("idx", [P, IDXW], i16, kind="ExternalInput")
    ident_d = nc.dram_tensor("ident", [P, P], bf16, kind="ExternalInput")
    val_d = nc.dram_tensor("val", [P, S], bf16, kind="ExternalInput")
    w_d = nc.dram_tensor("w", [v, o], wdt, kind="ExternalInput")
    odt = bf16 if OUT_BF16 else f32
    out_d = nc.dram_tensor("out", [tiles * P, o], odt, kind="ExternalOutput")

    from contextlib import ExitStack

    with tile.TileContext(nc) as tc:
        with ExitStack() as stack:
            io = stack.enter_context(tc.tile_pool(name="io", bufs=1))
            gp = stack.enter_context(tc.tile_pool(name="gp", bufs=GBUFS))
            dp = stack.enter_context(tc.tile_pool(name="dp", bufs=3))
            ob = stack.enter_context(tc.tile_pool(name="ob", bufs=3))
            ps = stack.enter_context(
                tc.tile_pool(name="ps", bufs=3, space="PSUM"))
            ap2 = stack.enter_context(tc.tile_pool(name="ap2", bufs=4))
            idx_sb = io.tile([P, IDXW], i16)
            # split the idx load so the first calls' indices land first
            cut = min(IDXW, 1024)
            nc.sync.dma_start(out=idx_sb[:, 0:cut], in_=idx_d.ap()[:, 0:cut])
            if cut < IDXW:
                nc.sync.dma_start(out=idx_sb[:, cut:IDXW],
                                  in_=idx_d.ap()[:, cut:IDXW])
            val_sb = io.tile([P, S], bf16)
            nc.sync.dma_start(out=val_sb[:], in_=val_d.ap())
            ident_sb = io.tile([P, P], bf16)
            nc.sync.dma_start(out=ident_sb[:], in_=ident_d.ap())
            # fp32 copy of vals: DVE tensor_scalar ops need fp32 scalars
            val32_sb = io.tile([P, S], f32)
            nc.vector.tensor_copy(out=val32_sb[:], in_=val_sb[:])

            wsrc = (w_d.ap()[ABASE:AEND, :], w_d.ap()[BBASE:v, :])
            ibase = (0, SA * 8)

            # HAM warm-up: throwaway matmuls bridge the initial gather fill
            # so the PE reaches K=8/8 (2.4GHz) before real MMs start.
            warm_ps = ps.tile([P, o], f32, tag="warm")
            for _ in range(NWARM):
                nc.tensor.matmul(out=warm_ps[:], lhsT=ident_sb[:],
                                 rhs=val_sb[:, 0:o], start=True, stop=True)

            nreg = {js: nc.gpsimd.to_reg(js * P)
                    for js in {c[2] for c in plan}}

            gts = [None] * len(plan)

            def emit_call(ci):
                s, off, js, _ = plan[ci]
                gt = gp.tile([P, js, o], wdt, tag="g")
                nc.gpsimd.dma_gather(
                    gt[:],
                    wsrc[s],
                    idx_sb[:, ibase[s] + off * 8: ibase[s] + (off + js) * 8],
                    js * P,
                    nreg[js],
                    o,
                    queue_num=ci % NQ,
                    single_packet=(js * P <= 1024),
                )
                gts[ci] = gt

            soff = 0
            aoff = boff = 0  # consumed slots per stream
            for g in range(tiles):
                Jt = JA[g] + JB[g]
                # emit every call whose first-served tile is g
                for ci, c in enumerate(plan):
                    if c[3] == g and gts[ci] is None:
                        emit_call(ci)

                Jp = Jt - JD  # PE slots; last JD slots ride the DVE FMA
                d = dp.tile([P, Jp, P], bf16, tag="d")
                nc.vector.tensor_tensor(
                    out=d[:],
                    in0=val_sb[:, soff:soff + Jp].unsqueeze(2).to_broadcast(
                        [P, Jp, P]),
                    in1=ident_sb[:].unsqueeze(1).to_broadcast([P, Jp, P]),
                    op=mybir.AluOpType.mult,
                )

                def slot_ref(k):
                    if k < JA[g]:
                        return slot2call[(0, aoff + k)]
                    return slot2call[(1, boff + k - JA[g])]

                psum = ps.tile([P, o], f32)
                for k in range(Jp):
                    ci, j = slot_ref(k)
                    nc.tensor.matmul(
                        out=psum[:],
                        lhsT=d[:, k:k + 1, :],
                        rhs=gts[ci][:, j:j + 1, :],
                        start=(k == 0),
                        stop=(k == Jp - 1),
                    )

                # DVE path: acc = sum val_k * row_k over the last JD slots
                acc = None
                for k in range(Jp, Jt):
                    ci, j = slot_ref(k)
                    nacc = ap2.tile([P, o], fp16, tag="a")
                    if acc is None:
                        nc.vector.tensor_scalar_mul(
                            out=nacc[:],
                            in0=gts[ci][:, j, :],
                            scalar1=val32_sb[:, soff + k:soff + k + 1],
                        )
                    else:
                        nc.vector.scalar_tensor_tensor(
                            out=nacc[:],
                            in0=gts[ci][:, j, :],
                            scalar=val32_sb[:, soff + k:soff + k + 1],
                            in1=acc[:],
                            op0=mybir.AluOpType.mult,
                            op1=mybir.AluOpType.add,
                        )
                    acc = nacc

                out_sb = ob.tile([P, o], odt, tag="o")
                nc.vector.tensor_tensor(
                    out=out_sb[:], in0=psum[:], in1=acc[:],
                    op=mybir.AluOpType.add,
                )
                nc.sync.dma_start(
                    out=out_d.ap()[g * P:(g + 1) * P, :], in_=out_sb[:],
                )
                aoff += JA[g]
                boff += JB[g]
                soff += Jt

    nc.compile()
    return nc


def host_prep(fi, fv, w, ncores=NCORES, wdt_name=WDT):
    """Split features between the two table windows, build per-core
    idx/val streams (tile-major A stream then B stream)."""
    fi = np.asarray(fi)
    fv = np.asarray(fv, dtype=np.float32)
    nrows, nf = fi.shape
    v, o = w.shape
    rows_per_core = nrows // ncores
    tiles = rows_per_core // P
    assert tiles * P * ncores == nrows

    valid = fi >= 0
    fvm = np.where(valid, fv, np.float32(0.0))
    idx = np.clip(fi, 0, v - 1).astype(np.int64)
    must_a = ((idx < BBASE) & valid) | ~valid
    must_b = idx >= AEND
    a_cnt = must_a.sum(axis=1)
    b_cnt = must_b.sum(axis=1)

    row_tile = (np.arange(nrows) % rows_per_core) // P
    JA, JB = [], []
    for g in range(tiles):
        m = row_tile == g
        maxa = int(a_cnt[m].max())
        maxb = int(b_cnt[m].max())
        T = max(nf, maxa + maxb)
        JA.append(T - maxb)
        JB.append(maxb)

    if wdt_name == "float8e3":
        w_dev = (w * np.float32(2.0**WEXP)).astype(ml_dtypes.float8_e3m4)
        fvm = fvm * np.float32(2.0**-WEXP)
    else:
        w_dev = w.astype(getattr(ml_dtypes, wdt_name))

    JA_pad = list(JA)
    if sum(JA_pad) % 2:
        JA_pad[-1] += 1
    SA, SB = sum(JA_pad), sum(JB)
    S = SA + SB
    IDXW = S * 8

    in_maps = []
    for c in range(ncores):
        idxA = np.zeros((P, SA), dtype=np.int16)
        idxB = np.zeros((P, SB), dtype=np.int16)
        val_stream = np.zeros((P, S), dtype=np.float32)
        aoff = boff = soff = 0
        for g in range(tiles):
            rows = slice(c * rows_per_core + g * P,
                         c * rows_per_core + (g + 1) * P)
            ridx = idx[rows]
            rval = fvm[rows]
            rma = must_a[rows]
            rmb = must_b[rows]
            jA, jB = JA_pad[g], JB[g]
            for p in range(P):
                ia = np.nonzero(rma[p])[0]
                ib = np.nonzero(rmb[p])[0]
                im = np.nonzero(~rma[p] & ~rmb[p])[0]
                na = min(len(im), jA - len(ia))
                a_feats = np.concatenate([ia, im[:na]])
                b_feats = np.concatenate([ib, im[na:]])
                idxA[p, aoff:aoff + len(a_feats)] = ridx[p, a_feats].astype(
                    np.int16)
                idxB[p, boff:boff + len(b_feats)] = (
                    ridx[p, b_feats] - BBASE).astype(np.int16)
                val_stream[p, soff:soff + len(a_feats)] = rval[p, a_feats]
                val_stream[p, soff + jA:soff + jA + len(b_feats)] = (
                    rval[p, b_feats])
            aoff += jA
            boff += jB
            soff += jA + jB
        # wrap each stream for the ucode idx layout: slot-major then [16, n*8]
        parts = []
        for m, n in ((idxA, SA), (idxB, SB)):
            flat = m.T.reshape(n * P)
            parts.append(flat.reshape(n * 8, 16).T)
        idx_stream = np.concatenate(parts, axis=1)  # [16, IDXW]
        in_maps.append({
            "idx": np.ascontiguousarray(np.tile(idx_stream, (8, 1))),
            "val": val_stream.astype(ml_dtypes.bfloat16),
            "w": w_dev,
            "ident": np.eye(P, dtype=ml_dtypes.bfloat16),
        })
    return tuple(JA), tuple(JB), in_maps


_nc_cache = {}


def _get_nc(JA, JB, wdt_name):
    key = (JA, JB, wdt_name, JSUB, GBUFS, NQ)
    if key not in _nc_cache:
        _nc_cache[key] = build_kernel(JA, JB, wdt_name)
    return _nc_cache[key]


def _ensure_ntff_hook():
    import sys
    import types
    if "antenv.axon_hooks" in sys.modules:
        return
    try:
        from trn_agent_boot.trn_boot import _ntff_profile_via_ctypes
        hook = _ntff_profile_via_ctypes("/opt/axon/libaxon_pjrt.so")
    except Exception:
        hook = None
    try:
        mod = types.ModuleType("antenv.axon_hooks")
        mod.get_axon_ntff_profile_hook = lambda: hook
        mod.set_axon_ntff_profile_hook = lambda h: None
        sys.modules["antenv.axon_hooks"] = mod
        import antenv
        antenv.axon_hooks = mod
    except Exception:
        pass
    try:
        from concourse import bass_utils
        bass_utils.upload_artifacts = lambda tmpdir: tmpdir
    except Exception:
        pass


def run_on_hw(feature_indices, feature_values, weight, bias, trace=False,
              wdt_name=WDT):
    from concourse import bass_utils
    _ensure_ntff_hook()
    w = np.ascontiguousarray(np.asarray(weight), dtype=np.float32)
    b = np.asarray(bias, dtype=np.float32).reshape(-1)
    JA, JB, in_maps = host_prep(
        feature_indices, feature_values, w, wdt_name=wdt_name)
    nc = _get_nc(JA, JB, wdt_name)
    res = bass_utils.run_bass_kernel_spmd(
        nc, in_maps, core_ids=list(range(NCORES)), trace=trace,
    )
    out = np.concatenate(
        [np.asarray(r["out"]).astype(np.float32) for r in res.results], axis=0)
    out = out + b[None, :]
    return out, res


def kernel(feature_indices, feature_values, weight, bias):
    out, _ = run_on_hw(feature_indices, feature_values, weight, bias,
                       trace=False)
    return out


# revision 14
# speedup vs baseline: 1.0285x; 1.0285x over previous
"""Trainium2 Bass kernel for FeatureTransformerSlice (embedding lookup), v3.

out[b, :] = bias + sum_f mask(idx[b,f]) * val[b,f] * weight[max(idx[b,f],0), :]

Strategy (8 NeuronCores, data-parallel over batch):
  - Per-core work: random gather of 2048*32 = 64Ki table rows + rank-1
    combines.  The binding resource is SWDGE descriptor generation on the
    GpSimd Q7 cores (~2.6 ns/row aggregate over the 4 SWDGE queues; each
    queue's descgen runs on its own Q7 core pair, cpu 2q/2q+1).  dma_gather
    calls are issued round-robin over all 4 queues; js=8 (1024 idxs) is the
    largest call that fits the 64-descriptor/engine single-packet limit.
  - The table is cast host-side to fp8 e3m4 ("float8e3", scaled by 2^11 so
    values sit in the normal range; the 2^-11 rides the bf16 vals exactly),
    halving SDMA drain time vs bf16 so descgen stays the only bottleneck.
    rel-err lands at 1.58e-2 (budget 2e-2), dominated by the 4-bit mantissa.
  - dma_gather indices are int16, so features are split between two
    overlapping table windows: A = w[0:32768] and B = w[8192:40960]
    (local idx - 8192).  Features in the overlap ride either window, so per
    tile J_B = max must-B count and J_A = 32 - J_B pack every row's 32
    features with zero padding.
  - v3 packs the gather calls ACROSS tile boundaries: all tiles' A-slots
    form one slot stream (tile-major), ditto B; calls take uniform js=8
    bites from each stream, so there are ~65 calls instead of 84 and no
    tiny fragment calls.  num_idxs registers are hoisted (no per-call MOVE).
  - Per batch tile: gathered [128, *, 512] fp8 rows are combined on PE as
    diag(val_j) matmuls (bf16 lhsT x fp8 rhs) accumulating in fp32 PSUM;
    Scalar evacuates PSUM as bf16; bias is added host-side.  ~48 warm-up
    matmuls bridge the initial gather fill so the PE reaches 2.4 GHz early.
"""

import numpy as np
import ml_dtypes

P = 128
B = 16384
F = 32
V = 40960
O = 512
NCORES = 8
ABASE = 0           # window A = rows [0, 32768)
BBASE = V - 32768   # window B = rows [8192, 40960)
AEND = 32768

WDT = "float8e3"          # device table dtype (e3m4; rows scaled by 2^WEXP)
WEXP = 11                 # w stored as w*2^WEXP (max |w|*2^11 ~ 10.1 < 15.5)
GBUFS = 32                # gather call ring depth
JSUB = 8                  # slots per dma_gather call (64 descs/engine limit)
OUT_BF16 = True           # device writes bf16 output; host upcasts
NQ = 4                    # SWDGE queues (max 4); round-robin
NWARM = 48                # PE warm-up matmuls
JD = 0                    # per-tile slots on DVE FMA (fp8-in DVE is ~4ns/elem: keep 0)


def _plan_calls(JA, JB):
    """Chunk the tile-major A and B slot streams into uniform js<=JSUB calls.

    Returns (callplan, slot2call, JA) where callplan is a list of
    (stream, stream_slot_off, js, t0) in emission order (grouped by the
    first tile each call serves), slot2call maps (stream, stream_slot) ->
    (call_index, j), and JA may have its last entry padded by one slot to
    keep the total A-slot count even (32B idx alignment for the B stream).
    """
    JA = list(JA)
    if sum(JA) % 2:
        JA[-1] += 1  # pad slot (idx 0, val 0) keeps B stream 32B-aligned
    tile_of = {0: [], 1: []}
    for g in range(len(JA)):
        tile_of[0] += [g] * JA[g]
        tile_of[1] += [g] * JB[g]
    calls = {}
    for s in (0, 1):
        n = len(tile_of[s])
        offs = list(range(0, n, JSUB))
        calls[s] = [(s, o, min(JSUB, n - o), tile_of[s][o]) for o in offs]
    # emission order: by first-served tile, A before B
    plan = sorted(calls[0] + calls[1], key=lambda c: (c[3], c[0]))
    slot2call = {}
    for ci, (s, o, js, _) in enumerate(plan):
        for j in range(js):
            slot2call[(s, o + j)] = (ci, j)
    return plan, slot2call, tuple(JA)


def build_kernel(JA, JB, wdt_name=WDT, v=V, o=O):
    import concourse.bacc as bacc
    import concourse.mybir as mybir
    import concourse.tile as tile

    f32 = mybir.dt.float32
    bf16 = mybir.dt.bfloat16
    fp16 = mybir.dt.float16
    i16 = mybir.dt.int16
    wdt = getattr(mybir.dt, wdt_name)
    tiles = len(JA)
    assert len(JB) == tiles

    plan, slot2call, JA = _plan_calls(JA, JB)
    SA, SB = sum(JA), sum(JB)
    S = SA + SB
    IDXW = S * 8  # int16 columns; A stream then B stream

    nc = bacc.Bacc("TRN2", target_bir_lowering=False, debug=False,
                   num_swdge_queues=NQ)

    idx_d = nc.dram_tensor("idx", [P, IDXW], i16, kind="ExternalInput")
    ident_d = nc.dram_tensor("ident", [P, P], bf16, kind="ExternalInput")
    val_d = nc.dram_tensor("val", [P, S], bf16, kind="ExternalInput")
    w_d = nc.dram_tensor("w", [v, o], wdt, kind="ExternalInput")
    odt = bf16 if OUT_BF16 else f32
    out_d = nc.dram_tensor("out", [tiles * P, o], odt, kind="ExternalOutput")

    from contextlib import ExitStack

    with tile.TileContext(nc) as tc:
        with ExitStack() as stack:
            io = stack.enter_context(tc.tile_pool(name="io", bufs=1))
            gp = stack.enter_context(tc.tile_pool(name="gp", bufs=GBUFS))
            dp = stack.enter_context(tc.tile_pool(name="dp", bufs=3))
            ob = stack.enter_context(tc.tile_pool(name="ob", bufs=3))
            ps = stack.enter_context(
                tc.tile_pool(name="ps", bufs=3, space="PSUM"))
            ap2 = stack.enter_context(tc.tile_pool(name="ap2", bufs=4))
            idx_sb = io.tile([P, IDXW], i16)
            # split the idx load so the first calls' indices land first
            cut = min(IDXW, 1024)
            nc.sync.dma_start(out=idx_sb[:, 0:cut], in_=idx_d.ap()[:, 0:cut])
            if cut < IDXW:
                nc.sync.dma_start(out=idx_sb[:, cut:IDXW],
                                  in_=idx_d.ap()[:, cut:IDXW])
            val_sb = io.tile([P, S], bf16)
            nc.sync.dma_start(out=val_sb[:], in_=val_d.ap())
            ident_sb = io.tile([P, P], bf16)
            nc.sync.dma_start(out=ident_sb[:], in_=ident_d.ap())
            # fp32 copy of vals: DVE tensor_scalar ops need fp32 scalars
            val32_sb = io.tile([P, S], f32)
            nc.vector.tensor_copy(out=val32_sb[:], in_=val_sb[:])

            wsrc = (w_d.ap()[ABASE:AEND, :], w_d.ap()[BBASE:v, :])
            ibase = (0, SA * 8)

            # HAM warm-up: throwaway matmuls bridge the initial gather fill
            # so the PE reaches K=8/8 (2.4GHz) before real MMs start.
            warm_ps = ps.tile([P, o], f32, tag="warm")
            for _ in range(NWARM):
                nc.tensor.matmul(out=warm_ps[:], lhsT=ident_sb[:],
                                 rhs=val_sb[:, 0:o], start=True, stop=True)

            nreg = {js: nc.gpsimd.to_reg(js * P)
                    for js in {c[2] for c in plan}}

            gts = [None] * len(plan)

            def emit_call(ci):
                s, off, js, _ = plan[ci]
                gt = gp.tile([P, js, o], wdt, tag="g")
                nc.gpsimd.dma_gather(
                    gt[:],
                    wsrc[s],
                    idx_sb[:, ibase[s] + off * 8: ibase[s] + (off + js) * 8],
                    js * P,
                    nreg[js],
                    o,
                    queue_num=ci % NQ,
                    single_packet=(js * P <= 1024),
                )
                gts[ci] = gt

            soff = 0
            aoff = boff = 0  # consumed slots per stream
            for g in range(tiles):
                Jt = JA[g] + JB[g]
                # emit every call whose first-served tile is g
                for ci, c in enumerate(plan):
                    if c[3] == g and gts[ci] is None:
                        emit_call(ci)

                Jp = Jt - JD  # PE slots; last JD slots ride the DVE FMA
                d = dp.tile([P, Jp, P], bf16, tag="d")
                nc.vector.tensor_tensor(
                    out=d[:],
                    in0=val_sb[:, soff:soff + Jp].unsqueeze(2).to_broadcast(
                        [P, Jp, P]),
                    in1=ident_sb[:].unsqueeze(1).to_broadcast([P, Jp, P]),
                    op=mybir.AluOpType.mult,
                )

                def slot_ref(k):
                    if k < JA[g]:
                        return slot2call[(0, aoff + k)]
                    return slot2call[(1, boff + k - JA[g])]

                psum = ps.tile([P, o], f32)
                for k in range(Jp):
                    ci, j = slot_ref(k)
                    nc.tensor.matmul(
                        out=psum[:],
                        lhsT=d[:, k:k + 1, :],
                        rhs=gts[ci][:, j:j + 1, :],
                        start=(k == 0),
                        stop=(k == Jp - 1),
                    )

                # DVE path: acc = sum val_k * row_k over the last JD slots
                acc = None
                for k in range(Jp, Jt):
                    ci, j = slot_ref(k)
                    nacc = ap2.tile([P, o], fp16, tag="a")
                    if acc is None:
                        nc.vector.tensor_scalar_mul(
                            out=nacc[:],
                            in0=gts[ci][:, j, :],
                            scalar1=val32_sb[:, soff + k:soff + k + 1],
                        )
                    else:
                        nc.vector.scalar_tensor_tensor(
                            out=nacc[:],
                            in0=gts[ci][:, j, :],
                            scalar=val32_sb[:, soff + k:soff + k + 1],
                            in1=acc[:],
                            op0=mybir.AluOpType.mult,
                            op1=mybir.AluOpType.add,
                        )
                    acc = nacc

                out_sb = ob.tile([P, o], odt, tag="o")
                if acc is None:
                    nc.scalar.copy(out=out_sb[:], in_=psum[:])
                else:
                    nc.vector.tensor_tensor(
                        out=out_sb[:], in0=psum[:], in1=acc[:],
                        op=mybir.AluOpType.add,
                    )
                nc.sync.dma_start(
                    out=out_d.ap()[g * P:(g + 1) * P, :], in_=out_sb[:],
                )
                aoff += JA[g]
                boff += JB[g]
                soff += Jt

    nc.compile()
    return nc


def host_prep(fi, fv, w, ncores=NCORES, wdt_name=WDT):
    """Split features between the two table windows, build per-core
    idx/val streams (tile-major A stream then B stream)."""
    fi = np.asarray(fi)
    fv = np.asarray(fv, dtype=np.float32)
    nrows, nf = fi.shape
    v, o = w.shape
    rows_per_core = nrows // ncores
    tiles = rows_per_core // P
    assert tiles * P * ncores == nrows

    valid = fi >= 0
    fvm = np.where(valid, fv, np.float32(0.0))
    idx = np.clip(fi, 0, v - 1).astype(np.int64)
    must_a = ((idx < BBASE) & valid) | ~valid
    must_b = idx >= AEND
    a_cnt = must_a.sum(axis=1)
    b_cnt = must_b.sum(axis=1)

    row_tile = (np.arange(nrows) % rows_per_core) // P
    JA, JB = [], []
    for g in range(tiles):
        m = row_tile == g
        maxa = int(a_cnt[m].max())
        maxb = int(b_cnt[m].max())
        T = max(nf, maxa + maxb)
        JA.append(T - maxb)
        JB.append(maxb)

    if wdt_name == "float8e3":
        w_dev = (w * np.float32(2.0**WEXP)).astype(ml_dtypes.float8_e3m4)
        fvm = fvm * np.float32(2.0**-WEXP)
    else:
        w_dev = w.astype(getattr(ml_dtypes, wdt_name))

    JA_pad = list(JA)
    if sum(JA_pad) % 2:
        JA_pad[-1] += 1
    SA, SB = sum(JA_pad), sum(JB)
    S = SA + SB
    IDXW = S * 8

    in_maps = []
    for c in range(ncores):
        idxA = np.zeros((P, SA), dtype=np.int16)
        idxB = np.zeros((P, SB), dtype=np.int16)
        val_stream = np.zeros((P, S), dtype=np.float32)
        aoff = boff = soff = 0
        for g in range(tiles):
            rows = slice(c * rows_per_core + g * P,
                         c * rows_per_core + (g + 1) * P)
            ridx = idx[rows]
            rval = fvm[rows]
            rma = must_a[rows]
            rmb = must_b[rows]
            jA, jB = JA_pad[g], JB[g]
            for p in range(P):
                ia = np.nonzero(rma[p])[0]
                ib = np.nonzero(rmb[p])[0]
                im = np.nonzero(~rma[p] & ~rmb[p])[0]
                na = min(len(im), jA - len(ia))
                a_feats = np.concatenate([ia, im[:na]])
                b_feats = np.concatenate([ib, im[na:]])
                idxA[p, aoff:aoff + len(a_feats)] = ridx[p, a_feats].astype(
                    np.int16)
                idxB[p, boff:boff + len(b_feats)] = (
                    ridx[p, b_feats] - BBASE).astype(np.int16)
                val_stream[p, soff:soff + len(a_feats)] = rval[p, a_feats]
                val_stream[p, soff + jA:soff + jA + len(b_feats)] = (
                    rval[p, b_feats])
            aoff += jA
            boff += jB
            soff += jA + jB
        # wrap each stream for the ucode idx layout: slot-major then [16, n*8]
        parts = []
        for m, n in ((idxA, SA), (idxB, SB)):
            flat = m.T.reshape(n * P)
            parts.append(flat.reshape(n * 8, 16).T)
        idx_stream = np.concatenate(parts, axis=1)  # [16, IDXW]
        in_maps.append({
            "idx": np.ascontiguousarray(np.tile(idx_stream, (8, 1))),
            "val": val_stream.astype(ml_dtypes.bfloat16),
            "w": w_dev,
            "ident": np.eye(P, dtype=ml_dtypes.bfloat16),
        })
    return tuple(JA), tuple(JB), in_maps


_nc_cache = {}


def _get_nc(JA, JB, wdt_name):
    key = (JA, JB, wdt_name, JSUB, GBUFS, NQ)
    if key not in _nc_cache:
        _nc_cache[key] = build_kernel(JA, JB, wdt_name)
    return _nc_cache[key]


def _ensure_ntff_hook():
    import sys
    import types
    if "antenv.axon_hooks" in sys.modules:
        return
    try:
        from trn_agent_boot.trn_boot import _ntff_profile_via_ctypes
        hook = _ntff_profile_via_ctypes("/opt/axon/libaxon_pjrt.so")
    except Exception:
        hook = None
    try:
        mod = types.ModuleType("antenv.axon_hooks")
        mod.get_axon_ntff_profile_hook = lambda: hook
        mod.set_axon_ntff_profile_hook = lambda h: None
        sys.modules["antenv.axon_hooks"] = mod
        import antenv
        antenv.axon_hooks = mod
    except Exception:
        pass
    try:
        from concourse import bass_utils
        bass_utils.upload_artifacts = lambda tmpdir: tmpdir
    except Exception:
        pass


def run_on_hw(feature_indices, feature_values, weight, bias, trace=False,
              wdt_name=WDT):
    from concourse import bass_utils
    _ensure_ntff_hook()
    w = np.ascontiguousarray(np.asarray(weight), dtype=np.float32)
    b = np.asarray(bias, dtype=np.float32).reshape(-1)
    JA, JB, in_maps = host_prep(
        feature_indices, feature_values, w, wdt_name=wdt_name)
    nc = _get_nc(JA, JB, wdt_name)
    res = bass_utils.run_bass_kernel_spmd(
        nc, in_maps, core_ids=list(range(NCORES)), trace=trace,
    )
    out = np.concatenate(
        [np.asarray(r["out"]).astype(np.float32) for r in res.results], axis=0)
    out = out + b[None, :]
    return out, res


def kernel(feature_indices, feature_values, weight, bias):
    out, _ = run_on_hw(feature_indices, feature_values, weight, bias,
                       trace=False)
    return out
